# revision 1
# baseline (speedup 1.0000x reference)
"""Causal self-attention (RoPE) Trainium2 Bass kernel, 8-core SPMD.

Sharding: core c -> batch c//2, head-group c%2 (8 of 16 heads).
Per core: q/k/v projections column-sharded over heads, block-causal attention
for its 8 heads, out-projection row-sharded; the host sums the two partial
outputs per batch and adds bo.

All matmuls run as float32r (single-pass fp32_mode=HIGH, ~1.5e-4 component
error). Scores are computed in s^T [kv, q] layout so softmax denominators come
from an ones-column in v and the attention output lands d-major for the
out-projection. The causal mask is applied by accumulating (-1000*I) @ Vmask
into the score PSUM; exp then underflows the masked lanes to zero.

PE-HAM note: matmuls whose stationary operand covers fewer than 128 partition
rows never lift the PE clock gate out of its cold 1.2 GHz state (measured).
The qk matmuls contract over a 64-dim head, so q and k row-blocks are
duplicated across both partition halves (s comes out doubled; the 0.5 is
folded into exp's scale immediate), making every hot-loop matmul full-K.
"""
import sys

sys.path.insert(0, "/opt/trn_rl_repo")

import math
from contextlib import ExitStack

import ml_dtypes
import numpy as np

import concourse.bass as bass
import concourse.tile as tile
from concourse import bacc, mybir
from concourse.bass_utils import run_bass_kernel_spmd

F32 = mybir.dt.float32
F32R = mybir.dt.float32r
BF16 = mybir.dt.bfloat16
AF = mybir.ActivationFunctionType

N_CORES = 8
B, T, D = 4, 2048, 1024
H, HD = 16, 64          # total heads, head dim
HC = 8                  # heads per core
DC = HC * HD            # 512 sharded projection dims per core
BASE = 10000
NT = T // 128           # 16 t-tiles
NM = DC // 128          # 4 m-tiles of q/k (2 heads each)
NK = D // 128           # 8 contraction tiles of D
NQC = T // 512          # 4 q-chunks per head
VW = HC * (HD + 1)      # 520: v_ext width (64 dims + ones col per head)


def _build_program():
    nc = bacc.Bacc("TRN2", target_bir_lowering=False, debug=False,
                   num_devices=N_CORES)

    def din(name, shape, dt=F32R):
        return nc.dram_tensor(name, shape, dt, kind="ExternalInput").ap()

    xT = din("xT", [D, T], BF16)              # x[b].T
    wqT = din("wqT", [D, DC], BF16)           # (Wq/8)[rows].T
    wkT = din("wkT", [D, DC], BF16)
    wvT = din("wvT", [D, DC], BF16)
    woT = din("woT", [DC, D])                 # Wo[:, rows].T
    bqk_cols = din("bqk_cols", [128, 8], F32)  # q bias m-cols 0-3, k bias 4-7
    bv_row = din("bv_row", [1, DC], BF16)
    ones_row = din("ones_row", [1, 512], BF16)  # constant ones
    ones_col = din("ones_col", [128, 8], F32)  # ones block for v_ext columns
    cosS = din("cosS", [128, T], BF16)        # 2-head-stacked cos table
    sinS = din("sinS", [128, T], BF16)        # sign-folded sin table
    vmask = din("vmask", [4, 128, 512], BF16)  # causal mask indicator per kv offset
    negI = din("negI", [128, 128], BF16)      # -1000 * I
    outT = nc.dram_tensor("outT", [D, T], F32, kind="ExternalOutput").ap()

    with tile.TileContext(nc) as tc, ExitStack() as top:
        p_const = top.enter_context(tc.tile_pool(name="const", bufs=1))
        ones_t = p_const.tile([1, 128], BF16, name="ones_t")
        nc.sync.dma_start(ones_t[:], ones_row[:, 0:128])
        bqk_t = p_const.tile([128, 8], F32, name="bqk_t")
        nc.sync.dma_start(bqk_t[:], bqk_cols[:])
        bv_t = p_const.tile([1, DC], BF16, name="bv_t")
        nc.sync.dma_start(bv_t[:], bv_row[:])
        ones_col_t = p_const.tile([128, 8], F32, name="ones_col_t")
        nc.sync.dma_start(ones_col_t[:], ones_col[:])
        vm = []
        for r in range(4):
            t = p_const.tile([128, 512], BF16, name=f"vm{r}")
            nc.scalar.dma_start(t[:], vmask[r])
            vm.append(t)
        negI_t = p_const.tile([128, 128], BF16, name="negI_t")
        nc.scalar.dma_start(negI_t[:], negI[:])

        # qT/kT live from the projection phase through attention; kT is reused
        # as the normalized attention output (yn) feeding the out-projection.
        p_qk = top.enter_context(tc.tile_pool(name="qk", bufs=1))
        qT = [p_qk.tile([128, T], BF16, name=f"qT{m}") for m in range(NM)]
        kT = [p_qk.tile([128, T], BF16, name=f"kT{m}") for m in range(NM)]
        p_yn = top.enter_context(tc.tile_pool(name="yn", bufs=1))
        yn = [p_yn.tile([128, T], F32R, name=f"yn{m}") for m in range(NM)]

        # ---- Phase QKV: q/k/v projections + rope, quarter-streamed, fused --
        p_v = top.enter_context(tc.tile_pool(name="pv_ext", bufs=1))
        v_ext = [p_v.tile([128, VW], F32R, name=f"vext{tt}") for tt in range(NT)]
        with tc.tile_pool(name="pqk_w", bufs=1) as p_w, \
             tc.tile_pool(name="pqk_tab", bufs=2) as p_tab, \
             tc.tile_pool(name="pqk_x", bufs=2) as p_x, \
             tc.tile_pool(name="pqk_tmp", bufs=3) as p_tmp, \
             tc.tile_pool(name="pqk_ps", bufs=5, space="PSUM") as ps_p, \
             tc.tile_pool(name="pqk_psv", bufs=3, space="PSUM") as ps_pv:
            wq, wk, wv = [], [], []
            for k in range(NK):
                wt = p_w.tile([128, DC], BF16, name=f"wq{k}")
                nc.sync.dma_start(wt[:], wqT[bass.ts(k, 128), :])
                wq.append(wt)
                wt = p_w.tile([128, DC], BF16, name=f"wk{k}")
                nc.sync.dma_start(wt[:], wkT[bass.ts(k, 128), :])
                wk.append(wt)
                wt = p_w.tile([128, DC], BF16, name=f"wv{k}")
                nc.sync.dma_start(wt[:], wvT[bass.ts(k, 128), :])
                wv.append(wt)
            for qtr in range(4):
                hs = bass.ts(qtr, 512)
                cos_t = p_tab.tile([128, 512], BF16, name="cos_t",
                                   tag="cos_t", bufs=2)
                nc.sync.dma_start(cos_t[:], cosS[:, hs])
                sin_t = p_tab.tile([128, 512], BF16, name="sin_t",
                                   tag="sin_t", bufs=2)
                nc.sync.dma_start(sin_t[:], sinS[:, hs])
                xs = []
                for k in range(NK):
                    xt = p_x.tile([128, 512], BF16, name=f"xs{k}",
                                  tag=f"xs{k}", bufs=2)
                    nc.sync.dma_start(xt[:], xT[bass.ts(k, 128), hs])
                    xs.append(xt)
                for m in range(NM):
                    for wi, (wlist, dest) in enumerate(((wq, qT), (wk, kT))):
                        acc = ps_p.tile([128, 512], F32, name="acc_p",
                                        tag="acc_p", bufs=5)
                        for k in range(NK):
                            nc.tensor.matmul(acc[:],
                                             wlist[k][:, bass.ts(m, 128)],
                                             xs[k][:],
                                             start=(k == 0),
                                             stop=(k == NK - 1))
                        qb = p_tmp.tile([128, 512], BF16, name="rope_qb",
                                        tag="rope_qb", bufs=3)
                        nc.scalar.activation(
                            qb[:], acc[:], AF.Identity,
                            bias=bqk_t[:, 4 * wi + m:4 * wi + m + 1])
                        shuf = p_tmp.tile([128, 512], BF16, name="rope_shuf",
                                          tag="rope_shuf", bufs=3)
                        for (dst, src) in ((0, 32), (32, 0), (64, 96),
                                           (96, 64)):
                            nc.vector.tensor_copy(shuf[dst:dst + 32, :],
                                                  qb[src:src + 32, :])
                        t1 = p_tmp.tile([128, 512], BF16, name="rope_t1",
                                        tag="rope_t1", bufs=3)
                        nc.vector.tensor_mul(t1[:], qb[:], cos_t[:])
                        nc.vector.tensor_mul(shuf[:], shuf[:], sin_t[:])
                        nc.vector.tensor_add(dest[m][:, hs], t1[:], shuf[:])
                for tl in range(4):
                    tt = 4 * qtr + tl
                    acc = ps_pv.tile([128, DC], F32, name="acc_v",
                                     tag="acc_v", bufs=3)
                    for k in range(NK):
                        nc.tensor.matmul(acc[:], xs[k][:, bass.ts(tl, 128)],
                                         wv[k][:], start=(k == 0), stop=False)
                    nc.tensor.matmul(acc[:], ones_t[:], bv_t[:],
                                     start=False, stop=True)
                    v3 = v_ext[tt][:].rearrange("p (h w) -> p h w", w=HD + 1)
                    nc.vector.tensor_copy(
                        v3[:, :, HD:HD + 1],
                        ones_col_t[:].rearrange("p (h w) -> p h w", w=1))
                    nc.vector.tensor_copy(
                        v3[:, :, 0:HD],
                        acc[:].rearrange("p (h w) -> p h w", w=HD))

        # ---------------- Phase A: attention ----------------
        with tc.tile_pool(name="pa_dup", bufs=2) as p_dup, \
             tc.tile_pool(name="pa_pt", bufs=3) as p_pt, \
             tc.tile_pool(name="pa_y", bufs=8) as p_y, \
             tc.tile_pool(name="pa_z", bufs=2) as p_z, \
             tc.tile_pool(name="pa_s", bufs=1, space="PSUM") as ps_s, \
             tc.tile_pool(name="pa_yt", bufs=2, space="PSUM") as ps_yt:
            for m in range(NM):
                zall = p_z.tile([8, 512], F32, name="zall", tag="zall",
                                bufs=2)
                zrec = p_z.tile([8, 512], F32, name="zrec", tag="zrec",
                                bufs=2)
                ysbs = []
                for hh in range(2):
                    h = 2 * m + hh
                    prow = slice(64 * hh, 64 * hh + 64)
                    # duplicate head rows across both partition halves so
                    # the qk stationary covers all 128 rows (HAM-warm)
                    kTd = p_dup.tile([128, T], BF16, name="kTd",
                                     tag="kTd", bufs=2)
                    qTd = p_dup.tile([128, T], BF16, name="qTd",
                                     tag="qTd", bufs=2)
                    for half in range(2):
                        nc.gpsimd.dma_start(kTd[bass.ts(half, 64), :],
                                            kT[m][prow, :])
                        nc.gpsimd.dma_start(qTd[bass.ts(half, 64), :],
                                            qT[m][prow, :])
                    for J in range(NQC):
                        yt = ps_yt.tile([65, 512], F32, name="yt",
                                        tag="yt", bufs=2)
                        nkv = 4 * J + 4
                        for kvp in range(nkv // 4):
                            squad = ps_s.tile([128, 2048], F32, name="squad",
                                              tag="squad", bufs=1)
                            for half in range(4):
                                kvt = 4 * kvp + half
                                sl = squad[:, bass.ts(half, 512)]
                                nc.tensor.matmul(
                                    sl, kTd[:, bass.ts(kvt, 128)],
                                    qTd[:, bass.ts(J, 512)],
                                    start=True, stop=(kvt < 4 * J))
                                if kvt >= 4 * J:
                                    nc.tensor.matmul(
                                        sl, negI_t[:], vm[kvt - 4 * J][:],
                                        start=False, stop=True)
                            pt = p_pt.tile([128, 2048], F32R, name="pt",
                                           tag="pt", bufs=3)
                            nc.scalar.activation(pt[:], squad[:], AF.Exp,
                                                 scale=0.5)
                            for half in range(4):
                                kvt = 4 * kvp + half
                                v3 = v_ext[kvt][:].rearrange(
                                    "p (h w) -> p h w", w=HD + 1)
                                nc.tensor.matmul(
                                    yt[:], v3[:, h, :],
                                    pt[:, bass.ts(half, 512)],
                                    start=(kvt == 0),
                                    stop=(kvt == nkv - 1))
                        ysb = p_y.tile([65, 512], F32, name="ysb",
                                       tag="ysb", bufs=8)
                        nc.vector.tensor_copy(ysb[:], yt[:])
                        i = hh * NQC + J
                        nc.sync.dma_start(zall[i:i + 1, :], ysb[64:65, :])
                        ysbs.append((hh, J, ysb))
                nc.vector.reciprocal(zrec[:], zall[:])
                for (hh, J, ysb) in ysbs:
                    i = hh * NQC + J
                    zr1 = p_z.tile([1, 512], F32, name="zr1", tag="zr1",
                                   bufs=2)
                    nc.sync.dma_start(zr1[:], zrec[i:i + 1, :])
                    zb = p_z.tile([64, 512], F32, name="zb", tag="zb",
                                  bufs=2)
                    nc.gpsimd.partition_broadcast(zb[:], zr1[:])
                    nc.vector.tensor_mul(
                        yn[m][64 * hh:64 * hh + 64, bass.ts(J, 512)],
                        ysb[0:64, :], zb[:])

        # ---------------- Phase O: out projection ----------------
        with tc.tile_pool(name="po_w", bufs=1) as p_wo, \
             tc.tile_pool(name="po_st", bufs=2) as p_st, \
             tc.tile_pool(name="po_ps", bufs=4, space="PSUM") as ps_o:
            wo = []
            for k in range(NM):
                wt = p_wo.tile([128, D], F32R, name=f"wo{k}")
                nc.sync.dma_start(wt[:], woT[bass.ts(k, 128), :])
                wo.append(wt)
            for M in range(NK):
                st = p_st.tile([128, T], F32, name="out_st", tag="out_st",
                               bufs=2)
                for n in range(NQC):
                    acc = ps_o.tile([128, 512], F32, name="acc_o", tag="acc_o",
                                    bufs=4)
                    for k in range(NM):
                        nc.tensor.matmul(acc[:], wo[k][:, bass.ts(M, 128)],
                                         yn[k][:, bass.ts(n, 512)],
                                         start=(k == 0), stop=(k == NM - 1))
                    nc.vector.tensor_copy(st[:, bass.ts(n, 512)], acc[:])
                nc.scalar.dma_start(outT[bass.ts(M, 128), :], st[:])

    nc.compile()
    return nc


_NC_CACHE = None


def _get_program():
    global _NC_CACHE
    if _NC_CACHE is None:
        _NC_CACHE = _build_program()
    return _NC_CACHE


def _host_inputs(x, Wq, bq, Wk, bk, Wv, bv, Wo, bo):
    scale = 1.0 / math.sqrt(HD)
    Wq_s = (np.asarray(Wq, dtype=np.float32) * scale).astype(np.float32)
    bq_s = (np.asarray(bq, dtype=np.float32) * scale).astype(np.float32)
    x = np.asarray(x, dtype=np.float32)
    Wk = np.asarray(Wk, dtype=np.float32)
    Wv = np.asarray(Wv, dtype=np.float32)
    Wo = np.asarray(Wo, dtype=np.float32)
    bk = np.asarray(bk, dtype=np.float32)
    bv = np.asarray(bv, dtype=np.float32)

    # rope tables, 2-head-stacked [128, T]
    j = np.arange(HD // 2, dtype=np.float64)
    theta = BASE ** (-2.0 * j / HD)                      # [32]
    pos = np.arange(1, T + 1, dtype=np.float64)          # [T]
    ang = pos[None, :] * theta[:, None]                  # [32, T]
    cos32 = np.cos(ang)
    sin32 = np.sin(ang)
    cos64 = np.concatenate([cos32, cos32], axis=0)       # [64, T]
    sin64 = np.concatenate([-sin32, sin32], axis=0)      # sign-folded
    cosS = np.concatenate([cos64, cos64], axis=0).astype(np.float32)
    sinS = np.concatenate([sin64, sin64], axis=0).astype(np.float32)

    p = np.arange(128)
    f = np.arange(512)
    vmask = np.zeros((4, 128, 512), dtype=np.float32)
    for r in range(4):
        vmask[r] = ((128 * r + p[:, None]) > f[None, :]).astype(np.float32)
    negI = (-1000.0 * np.eye(128)).astype(np.float32)
    ones_row = np.ones((1, 512), dtype=np.float32)

    in_maps = []
    for c in range(N_CORES):
        b, g = c // 2, c % 2
        rows = slice(DC * g, DC * (g + 1))
        bqk = np.zeros((128, 8), dtype=np.float32)
        for m in range(NM):
            bqk[:, m] = bq_s[rows][128 * m:128 * (m + 1)]
            bqk[:, 4 + m] = bk[rows][128 * m:128 * (m + 1)]
        bf = ml_dtypes.bfloat16
        in_maps.append({
            "xT": np.ascontiguousarray(x[b].T).astype(bf),
            "wqT": np.ascontiguousarray(Wq_s[rows].T).astype(bf),
            "wkT": np.ascontiguousarray(Wk[rows].T).astype(bf),
            "wvT": np.ascontiguousarray(Wv[rows].T).astype(bf),
            "woT": np.ascontiguousarray(Wo[:, rows].T),
            "bqk_cols": bqk,
            "bv_row": bv[rows].reshape(1, DC).astype(bf),
            "ones_row": ones_row.astype(bf),
            "ones_col": np.ones((128, 8), dtype=np.float32),
            "cosS": cosS.astype(bf),
            "sinS": sinS.astype(bf),
            "vmask": vmask.astype(bf),
            "negI": negI.astype(bf),
        })
    return in_maps


def kernel(x, Wq, bq, Wk, bk, Wv, bv, Wo, bo, _trace=False):
    nc = _get_program()
    in_maps = _host_inputs(x, Wq, bq, Wk, bk, Wv, bv, Wo, bo)
    res = run_bass_kernel_spmd(nc, in_maps, list(range(N_CORES)), trace=_trace)
    kernel.last_exec_time_ns = res.exec_time_ns
    bo = np.asarray(bo, dtype=np.float32)
    out = np.zeros((B, T, D), dtype=np.float32)
    for b in range(B):
        acc = res.results[2 * b]["outT"].astype(np.float32) + \
            res.results[2 * b + 1]["outT"].astype(np.float32)
        out[b] = acc.T + bo[None, :]
    return out



# revision 10
# speedup vs baseline: 1.5010x; 1.5010x over previous
"""Causal self-attention (RoPE) Trainium2 Bass kernel, 8-core SPMD.

Sharding: core c -> batch c//2, head-group c%2 (8 of 16 heads).
Per core: q/k/v projections column-sharded over heads, block-causal attention
for its 8 heads, out-projection row-sharded; the host sums the two partial
outputs per batch and adds bo.

Attention is software-pipelined over kv-tile DUOS: per (head, 512-q chunk),
each duo's two score matmuls land in a [128, 1024] PSUM tile (s^T [kv, q]
layout, bufs=3), exp follows on the scalar engine, the causal mask is a 0/1
indicator MULTIPLY on pt after exp (DVE, diag duos only), and the pv matmuls
trail two duos behind so the PE never waits on the exp chain. Softmax
denominators come from a ones-column in v_ext (yt row 64); normalize reads yt
straight out of PSUM (reciprocal on DVE, partition-broadcast on gpsimd),
deferred one chunk so the DVE queue never head-of-line blocks.

PE pstate note (measured): back-to-back matmuls pipeline at ~2.4 GHz with
~100ns/instr overhead; any PE idle gap restarts the clock ramp at 0.65 GHz.
q/k row-blocks are duplicated across both partition halves so the qk
stationary covers all 128 rows (s comes out doubled; the 0.5 is folded into
exp's scale immediate).
"""
import sys

sys.path.insert(0, "/opt/trn_rl_repo")

import math
from contextlib import ExitStack

import ml_dtypes
import numpy as np

import concourse.bass as bass
import concourse.tile as tile
from concourse import bacc, mybir
from concourse.bass_utils import run_bass_kernel_spmd

F32 = mybir.dt.float32
F32R = mybir.dt.float32r
BF16 = mybir.dt.bfloat16
AF = mybir.ActivationFunctionType

N_CORES = 8
B, T, D = 4, 2048, 1024
H, HD = 16, 64          # total heads, head dim
HC = 8                  # heads per core
DC = HC * HD            # 512 sharded projection dims per core
BASE = 10000
NT = T // 128           # 16 t-tiles
NM = DC // 128          # 4 m-tiles of q/k (2 heads each)
NK = D // 128           # 8 contraction tiles of D
VW = HC * (HD + 1)      # 520: v_ext width (64 dims + ones col per head)


def _build_program():
    nc = bacc.Bacc("TRN2", target_bir_lowering=False, debug=False,
                   num_devices=N_CORES)

    def din(name, shape, dt=F32R):
        return nc.dram_tensor(name, shape, dt, kind="ExternalInput").ap()

    xT = din("xT", [D, T], BF16)              # x[b].T
    wqT = din("wqT", [D, DC], BF16)           # (Wq/8)[rows].T
    wkT = din("wkT", [D, DC], BF16)
    wvT = din("wvT", [D, DC], BF16)
    woT = din("woT", [DC, D])                 # Wo[:, rows].T
    bqk_cols = din("bqk_cols", [128, 8], F32)  # q bias m-cols 0-3, k bias 4-7
    bv_row = din("bv_row", [1, DC], F32)
    ones_col = din("ones_col", [128, 8], F32)  # ones block for v_ext columns
    cosS = din("cosS", [128, T], BF16)        # 2-head-stacked cos table
    sinS = din("sinS", [128, T], BF16)        # sign-folded sin table
    vmask = din("vmask", [2, 128, 1024], F32)  # causal KEEP indicator, duo-wide
    outT = nc.dram_tensor("outT", [D, T], F32, kind="ExternalOutput").ap()

    # round-robin DMA issue over engine queues to parallelize HBM fetch
    dq = []

    def dma(dst, src):
        q = dq[0]
        dq.append(dq.pop(0))
        q.dma_start(dst, src)

    with tile.TileContext(nc) as tc, ExitStack() as top:
        dq.extend([nc.sync, nc.scalar, nc.gpsimd])
        p_const = top.enter_context(tc.tile_pool(name="const", bufs=1))
        bqk_t = p_const.tile([128, 8], F32, name="bqk_t")
        dma(bqk_t[:], bqk_cols[:])
        bv_t = p_const.tile([1, DC], F32, name="bv_t")
        dma(bv_t[:], bv_row[:])
        bvb = p_const.tile([128, DC], F32, name="bvb")
        nc.gpsimd.partition_broadcast(bvb[:], bv_t[:])
        ones_col_t = p_const.tile([128, 8], F32, name="ones_col_t")
        dma(ones_col_t[:], ones_col[:])
        vm = []
        for r in range(2):
            t = p_const.tile([128, 1024], F32, name=f"vm{r}")
            dma(t[:], vmask[r])
            vm.append(t)
        cos_t = p_const.tile([128, T], BF16, name="cos_t")
        dma(cos_t[:], cosS[:])
        sin_t = p_const.tile([128, T], BF16, name="sin_t")
        dma(sin_t[:], sinS[:])

        # qT/kT live from the projection phase through attention.
        p_qk = top.enter_context(tc.tile_pool(name="qk", bufs=1))
        qT = [p_qk.tile([128, T], BF16, name=f"qT{m}") for m in range(NM)]
        kT = [p_qk.tile([128, T], BF16, name=f"kT{m}") for m in range(NM)]
        p_yn = top.enter_context(tc.tile_pool(name="yn", bufs=1))
        yn = [p_yn.tile([128, T], F32R, name=f"yn{m}") for m in range(NM)]
        # out-projection weights: pool opened here so the DMAs can be issued
        # early (during attention) and the tiles survive into phase O.
        p_wo = top.enter_context(tc.tile_pool(name="po_w", bufs=1))

        # ---- Phase QKV: q/k/v projections + rope, fused ----
        p_v = top.enter_context(tc.tile_pool(name="pv_ext", bufs=1))
        v_ext = [p_v.tile([128, VW], F32R, name=f"vext{tt}") for tt in range(NT)]
        with tc.tile_pool(name="pqk_w", bufs=1) as p_w, \
             tc.tile_pool(name="pqk_x", bufs=1) as p_x, \
             tc.tile_pool(name="pqk_tmp", bufs=3) as p_tmp, \
             tc.tile_pool(name="pqk_ps", bufs=5, space="PSUM") as ps_p, \
             tc.tile_pool(name="pqk_psv", bufs=3, space="PSUM") as ps_pv:
            wq, wk, wv, xs = [], [], [], []
            for k in range(NK):
                xt = p_x.tile([128, T], BF16, name=f"xs{k}")
                dma(xt[:], xT[bass.ts(k, 128), :])
                xs.append(xt)
                wt = p_w.tile([128, DC], BF16, name=f"wq{k}")
                dma(wt[:], wqT[bass.ts(k, 128), :])
                wq.append(wt)
                wt = p_w.tile([128, DC], BF16, name=f"wk{k}")
                dma(wt[:], wkT[bass.ts(k, 128), :])
                wk.append(wt)
                wt = p_w.tile([128, DC], BF16, name=f"wv{k}")
                dma(wt[:], wvT[bass.ts(k, 128), :])
                wv.append(wt)
            for qtr in range(4):
                hs = bass.ts(qtr, 512)
                for m in range(NM):
                    for wi, (wlist, dest) in enumerate(((wq, qT), (wk, kT))):
                        acc = ps_p.tile([128, 512], F32, name="acc_p",
                                        tag="acc_p", bufs=5)
                        for k in range(NK):
                            nc.tensor.matmul(acc[:],
                                             wlist[k][:, bass.ts(m, 128)],
                                             xs[k][:, hs],
                                             start=(k == 0),
                                             stop=(k == NK - 1))
                        qb = p_tmp.tile([128, 512], BF16, name="rope_qb",
                                        tag="rope_qb", bufs=3)
                        nc.scalar.activation(
                            qb[:], acc[:], AF.Identity,
                            bias=bqk_t[:, 4 * wi + m:4 * wi + m + 1])
                        shuf = p_tmp.tile([128, 512], BF16, name="rope_shuf",
                                          tag="rope_shuf", bufs=3)
                        for (dst, src) in ((0, 32), (32, 0), (64, 96),
                                           (96, 64)):
                            nc.vector.tensor_copy(shuf[dst:dst + 32, :],
                                                  qb[src:src + 32, :])
                        t1 = p_tmp.tile([128, 512], BF16, name="rope_t1",
                                        tag="rope_t1", bufs=3)
                        nc.vector.tensor_mul(t1[:], qb[:], cos_t[:, hs])
                        nc.vector.tensor_mul(shuf[:], shuf[:], sin_t[:, hs])
                        nc.vector.tensor_add(dest[m][:, hs], t1[:], shuf[:])
                for tl in range(4):
                    tt = 4 * qtr + tl
                    acc = ps_pv.tile([128, DC], F32, name="acc_v",
                                     tag="acc_v", bufs=3)
                    for k in range(NK):
                        nc.tensor.matmul(acc[:], xs[k][:, bass.ts(tt, 128)],
                                         wv[k][:], start=(k == 0),
                                         stop=(k == NK - 1))
                    v3 = v_ext[tt][:].rearrange("p (h w) -> p h w", w=HD + 1)
                    nc.vector.tensor_copy(
                        v3[:, :, HD:HD + 1],
                        ones_col_t[:].rearrange("p (h w) -> p h w", w=1))
                    nc.vector.tensor_add(
                        v3[:, :, 0:HD],
                        acc[:].rearrange("p (h w) -> p h w", w=HD),
                        bvb[:].rearrange("p (h w) -> p h w", w=HD))

        # ---------------- Phase A: attention ----------------
        with tc.tile_pool(name="pa_dup", bufs=2) as p_dup, \
             tc.tile_pool(name="pa_pt", bufs=3) as p_pt, \
             tc.tile_pool(name="pa_z", bufs=2) as p_z, \
             tc.tile_pool(name="pa_s", bufs=2, space="PSUM") as ps_s, \
             tc.tile_pool(name="pa_yt", bufs=2, space="PSUM") as ps_yt:
            # prefetch out-projection weights while attention runs
            wo = []
            for k in range(NM):
                wt = p_wo.tile([128, D], F32R, name=f"wo{k}")
                nc.scalar.dma_start(wt[:], woT[bass.ts(k, 128), :])
                wo.append(wt)

            def dup_head(h):
                m, hh = h // 2, h % 2
                prow = slice(64 * hh, 64 * hh + 64)
                kTd = p_dup.tile([128, T], BF16, name="kTd", tag="kTd", bufs=2)
                qTd = p_dup.tile([128, T], BF16, name="qTd", tag="qTd", bufs=2)
                for half in range(2):
                    nc.sync.dma_start(kTd[bass.ts(half, 64), :], kT[m][prow, :])
                    nc.sync.dma_start(qTd[bass.ts(half, 64), :], qT[m][prow, :])
                return kTd, qTd

            dups = dup_head(0)
            next_dups = dup_head(1)
            pending_norm = None

            def flush_norm():
                nonlocal pending_norm
                if pending_norm is None:
                    return
                yt, m, hh, J = pending_norm
                zrec = p_z.tile([1, 512], F32, name="zrec", tag="zrec",
                                bufs=2)
                nc.vector.reciprocal(zrec[:], yt[64:65, :])
                zb = p_z.tile([64, 512], F32, name="zb", tag="zb", bufs=2)
                nc.gpsimd.partition_broadcast(zb[:], zrec[:])
                nc.vector.tensor_mul(
                    yn[m][64 * hh:64 * hh + 64, bass.ts(J, 512)],
                    yt[0:64, :], zb[:])
                pending_norm = None

            for h in range(HC):
                m, hh = h // 2, h % 2
                kTd, qTd = dups
                for J in range(4):
                    qs = bass.ts(J, 512)
                    yt = ps_yt.tile([65, 512], F32, name="yt", tag="yt",
                                    bufs=2)
                    pend = []

                    def emit_pv(ent):
                        d, pt = ent
                        for half in range(2):
                            kvt = 2 * d + half
                            v3 = v_ext[kvt][:].rearrange("p (h w) -> p h w",
                                                         w=HD + 1)
                            nc.tensor.matmul(
                                yt[:], v3[:, h, :],
                                pt[:, bass.ts(half, 512)],
                                start=(kvt == 0), stop=(kvt == 4 * J + 3))

                    for d in range(2 * J + 2):
                        sb = ps_s.tile([128, 1024], F32, name="sb", tag="sb",
                                       bufs=3)
                        for half in range(2):
                            nc.tensor.matmul(sb[:, bass.ts(half, 512)],
                                             kTd[:, bass.ts(2 * d + half, 128)],
                                             qTd[:, qs],
                                             start=True, stop=True)
                        pt = p_pt.tile([128, 1024], F32R, name="pt", tag="pt",
                                       bufs=3)
                        nc.scalar.activation(pt[:], sb[:], AF.Exp, scale=0.5)
                        if d >= 2 * J:
                            # diag duo: multiply in the 0/1 causal keep mask
                            nc.vector.tensor_mul(pt[:], pt[:],
                                                 vm[d - 2 * J][:])
                        pend.append((d, pt))
                        if len(pend) > 2:
                            emit_pv(pend.pop(0))
                    if J == 1 and h < HC - 1:
                        next_dups = dup_head(h + 1)
                    for ent in pend:
                        emit_pv(ent)
                    flush_norm()
                    pending_norm = (yt, m, hh, J)
                dups = next_dups
            flush_norm()

        # ---------------- Phase O: out projection ----------------
        with tc.tile_pool(name="po_st", bufs=2) as p_st, \
             tc.tile_pool(name="po_ps", bufs=4, space="PSUM") as ps_o:
            for M in range(NK):
                st = p_st.tile([128, T], F32, name="out_st", tag="out_st",
                               bufs=2)
                for n in range(4):
                    acc = ps_o.tile([128, 512], F32, name="acc_o", tag="acc_o",
                                    bufs=4)
                    for k in range(NM):
                        nc.tensor.matmul(acc[:], wo[k][:, bass.ts(M, 128)],
                                         yn[k][:, bass.ts(n, 512)],
                                         start=(k == 0), stop=(k == NM - 1))
                    nc.vector.tensor_copy(st[:, bass.ts(n, 512)], acc[:])
                nc.scalar.dma_start(outT[bass.ts(M, 128), :], st[:])

    nc.compile()
    return nc


_NC_CACHE = None


def _get_program():
    global _NC_CACHE
    if _NC_CACHE is None:
        _NC_CACHE = _build_program()
    return _NC_CACHE


def _host_inputs(x, Wq, bq, Wk, bk, Wv, bv, Wo, bo):
    scale = 1.0 / math.sqrt(HD)
    Wq_s = (np.asarray(Wq, dtype=np.float32) * scale).astype(np.float32)
    bq_s = (np.asarray(bq, dtype=np.float32) * scale).astype(np.float32)
    x = np.asarray(x, dtype=np.float32)
    Wk = np.asarray(Wk, dtype=np.float32)
    Wv = np.asarray(Wv, dtype=np.float32)
    Wo = np.asarray(Wo, dtype=np.float32)
    bk = np.asarray(bk, dtype=np.float32)
    bv = np.asarray(bv, dtype=np.float32)

    # rope tables, 2-head-stacked [128, T]
    j = np.arange(HD // 2, dtype=np.float64)
    theta = BASE ** (-2.0 * j / HD)                      # [32]
    pos = np.arange(1, T + 1, dtype=np.float64)          # [T]
    ang = pos[None, :] * theta[:, None]                  # [32, T]
    cos32 = np.cos(ang)
    sin32 = np.sin(ang)
    cos64 = np.concatenate([cos32, cos32], axis=0)       # [64, T]
    sin64 = np.concatenate([-sin32, sin32], axis=0)      # sign-folded
    cosS = np.concatenate([cos64, cos64], axis=0).astype(np.float32)
    sinS = np.concatenate([sin64, sin64], axis=0).astype(np.float32)

    p = np.arange(128)
    f = np.arange(512)
    vmask = np.zeros((2, 128, 1024), dtype=np.float32)
    for r in range(4):
        vmask[r // 2, :, 512 * (r % 2):512 * (r % 2) + 512] = (
            (128 * r + p[:, None]) <= f[None, :]).astype(np.float32)

    in_maps = []
    for c in range(N_CORES):
        b, g = c // 2, c % 2
        rows = slice(DC * g, DC * (g + 1))
        bqk = np.zeros((128, 8), dtype=np.float32)
        for m in range(NM):
            bqk[:, m] = bq_s[rows][128 * m:128 * (m + 1)]
            bqk[:, 4 + m] = bk[rows][128 * m:128 * (m + 1)]
        bf = ml_dtypes.bfloat16
        in_maps.append({
            "xT": np.ascontiguousarray(x[b].T).astype(bf),
            "wqT": np.ascontiguousarray(Wq_s[rows].T).astype(bf),
            "wkT": np.ascontiguousarray(Wk[rows].T).astype(bf),
            "wvT": np.ascontiguousarray(Wv[rows].T).astype(bf),
            "woT": np.ascontiguousarray(Wo[:, rows].T),
            "bqk_cols": bqk,
            "bv_row": bv[rows].reshape(1, DC),
            "ones_col": np.ones((128, 8), dtype=np.float32),
            "cosS": cosS.astype(bf),
            "sinS": sinS.astype(bf),
            "vmask": vmask,
        })
    return in_maps


def kernel(x, Wq, bq, Wk, bk, Wv, bv, Wo, bo, _trace=False):
    nc = _get_program()
    in_maps = _host_inputs(x, Wq, bq, Wk, bk, Wv, bv, Wo, bo)
    res = run_bass_kernel_spmd(nc, in_maps, list(range(N_CORES)), trace=_trace)
    kernel.last_exec_time_ns = res.exec_time_ns
    bo = np.asarray(bo, dtype=np.float32)
    out = np.zeros((B, T, D), dtype=np.float32)
    for b in range(B):
        acc = res.results[2 * b]["outT"].astype(np.float32) + \
            res.results[2 * b + 1]["outT"].astype(np.float32)
        out[b] = acc.T + bo[None, :]
    return out
